# revision 15
# baseline (speedup 1.0000x reference)
"""Fused conv3x3 -> GroupNorm(16) -> channel scale -> maxpool2x2 -> clamp[0,1]
Trainium2 Bass kernel, data-parallel over batch on 8 NeuronCores.

Input  x [32, 64, 128, 128] f32  -> output [32, 128, 63, 63] f32.
Each core handles 4 samples.

Conv: fp16 tap-pair matmuls, 5 PE passes per 8-row output group half
(vs 9 naive):
  - xt buffer: partition ci holds x[ci, row], ci+64 holds x[ci, row+1]
    -> one [128,128] stacked weight covers taps (kh0,kw)+(kh1,kw): 3 passes
  - xq buffer: both blocks hold x[ci, row+2], block1 shifted 1 column
    (loaded as flat row-major slices at +0/+1 element offsets, so both are
    single contiguous DMAs) -> taps (kh2,0)+(kh2,1) in 1 pass; (kh2,2)
    reads xq block0 at column offset 2 as a 64-contraction pass.
The two 64-contraction passes land on disjoint PE row groups so their
execution overlaps; PE busy ~157us/core is within ~20% of the MAC roofline.

Tail strategy (keeps the PE fed; DVE and ACT each stay under the PE's
~39us/sample):
  - stats WITHOUT bn_stats: the ACT PSUM-evacuation Copy produces per-group
    sum(y) via accum_out; one DVE tensor_tensor_reduce per group (y*y ->
    scratch, accum add) produces sum(y^2). Conv bias is folded analytically.
  - affine BEFORE pooling, as a single in-place DVE tensor_scalar over the
    whole sample -- contiguous fp16 SBUF operands hit the DVE 4x perf mode
    (~0.26 ns/el), so this is 3x cheaper than splitting affine over ACT+DVE.
  - single max-pool branch, vertical pairs first (contiguous innermost ->
    DVE 2x mode), then horizontal pairs (strided, 1x), then fused clamp.
  - per-sample tails (coeff chain / affine / pools / store) are emitted
    interleaved with the next sample's conv chunks so no queue head-blocks.
fp16 output upcast to f32 on host.
"""

import numpy as np

import concourse.bacc as bacc
import concourse.mybir as mybir
import concourse.tile as tile
from concourse.bass_utils import run_bass_kernel_spmd

N_CORES = 8
B_FULL, CIN, H, W = 32, 64, 128, 128
COUT = 128
BPC = B_FULL // N_CORES  # samples per core
OH = OW = 126
PH = PW = 63
NG = 16  # groups
GSZ = COUT // NG  # 8 channels per group
EPS = 1e-5
S = OH * OW  # spatial size per sample
NEL = float(S)  # elements per channel for stats

# (x_row0, n_xrows, out_row0, n_out_rows)
CHUNKS = [(0, 10, 0, 8), (8, 10, 8, 8), (16, 10, 16, 8), (24, 14, 24, 12),
          (36, 18, 36, 16), (52, 24, 52, 22), (74, 24, 74, 22), (96, 24, 96, 22),
          (118, 10, 118, 8)]
XROWS_MAX = 24
NGROUPS = 17  # total 8-or-smaller row groups per sample
USE_NEW_STATS = False  # sums via ACT accum_out + DVE ttr instead of bn_stats

_CACHED = {}


def _build():
    if "nc" in _CACHED:
        return _CACHED["nc"]
    f32 = mybir.dt.float32
    f16 = mybir.dt.float16
    AF = mybir.ActivationFunctionType
    OP = mybir.AluOpType

    nc = bacc.Bacc("TRN2", target_bir_lowering=False, debug=False)
    xs = nc.dram_tensor("xs", [BPC, CIN, H, W], f16, kind="ExternalInput").ap()
    wp_d = nc.dram_tensor("wp", [3, 128, COUT], f16, kind="ExternalInput").ap()
    w2_d = nc.dram_tensor("w2", [128, COUT], f16, kind="ExternalInput").ap()
    ws_d = nc.dram_tensor("ws", [128, COUT], f16, kind="ExternalInput").ap()
    cb_d = nc.dram_tensor("cb", [COUT, 1], f32, kind="ExternalInput").ap()
    gs_d = nc.dram_tensor("gs", [COUT, 1], f32, kind="ExternalInput").ap()
    gbs_d = nc.dram_tensor("gbs", [COUT, 1], f32, kind="ExternalInput").ap()
    bones_d = nc.dram_tensor("bones", [COUT, COUT], f32, kind="ExternalInput").ap()
    out_d = nc.dram_tensor("out", [BPC, COUT, PH, PW], f16, kind="ExternalOutput").ap()

    with tile.TileContext(nc) as tc:
        with (
            tc.tile_pool(name="consts", bufs=1) as cpool,
            tc.tile_pool(name="xpool", bufs=4) as xpool,
            tc.tile_pool(name="xqpool", bufs=4) as xqpool,
            tc.tile_pool(name="ypool", bufs=2) as ypool,
            tc.tile_pool(name="sqpool", bufs=1) as sqpool,
            tc.tile_pool(name="stpool", bufs=2) as stpool,
            tc.tile_pool(name="pvpool", bufs=2) as pvpool,
            tc.tile_pool(name="popool", bufs=2) as popool,
            tc.tile_pool(name="cps", bufs=3, space="PSUM") as cps,
            tc.tile_pool(name="gps", bufs=1, space="PSUM") as gps,
        ):
            wp = cpool.tile([128, 3 * COUT], f16, name="wp_t")
            w2 = cpool.tile([128, COUT], f16, name="w2_t")
            ws = cpool.tile([128, COUT], f16, name="ws_t")
            for kw in range(3):
                nc.sync.dma_start(wp[:, kw * COUT : (kw + 1) * COUT], wp_d[kw])
            nc.sync.dma_start(w2[:], w2_d[:])
            nc.sync.dma_start(ws[:], ws_d[:])
            cb = cpool.tile([COUT, 1], f32, name="cb_t")
            nc.sync.dma_start(cb[:], cb_d[:])
            gs = cpool.tile([COUT, 1], f32, name="gs_t")
            nc.sync.dma_start(gs[:], gs_d[:])
            gbs = cpool.tile([COUT, 1], f32, name="gbs_t")
            nc.sync.dma_start(gbs[:], gbs_d[:])
            bones = cpool.tile([COUT, COUT], f32, name="bones_t")
            nc.sync.dma_start(bones[:], bones_d[:])
            zeros1 = cpool.tile([COUT, 1], f32, name="zeros1")
            nc.vector.memset(zeros1[:], 0.0)
            # scratch for tensor_tensor_reduce main output (y^2, discarded)
            sqscr = sqpool.tile([128, 8 * OW], f16, name="sqscr")

            def tail_stats(tl):
                if not USE_NEW_STATS:
                    # baseline path: aggregate bn_stats
                    mv = stpool.tile([128, 2], f32, tag="mv", name="mv")
                    nc.vector.bn_aggr(mv[:], tl["st"][:])
                    st = stpool.tile([128, 2], f32, tag="sts", name="st")
                    nc.vector.tensor_tensor(st[:, 0:1], mv[:, 0:1], cb[:], OP.add)
                    t1sq = stpool.tile([128, 1], f32, tag="t1", name="t1sq")
                    nc.vector.tensor_tensor(t1sq[:], st[:, 0:1], st[:, 0:1], OP.mult)
                    nc.vector.tensor_tensor(st[:, 1:2], mv[:, 1:2], t1sq[:], OP.add)
                    tl["stv"] = st
                    return
                # per-channel E[y+cb] and E[(y+cb)^2] from raw sums
                s1c, sqc = tl["s1c"], tl["sqc"]
                S1 = stpool.tile([128, 1], f32, tag="S1", name="S1")
                nc.vector.tensor_reduce(S1[:], s1c[:], mybir.AxisListType.XYZW, OP.add)
                S2 = stpool.tile([128, 1], f32, tag="S2", name="S2")
                nc.vector.tensor_reduce(S2[:], sqc[:], mybir.AxisListType.XYZW, OP.add)
                st = stpool.tile([128, 2], f32, tag="sts", name="st")
                # st0 = S1/N + cb
                nc.vector.tensor_scalar(st[:, 0:1], S1[:], 1.0 / NEL, cb[:], OP.mult, OP.add)
                # st1 = S2/N + cb*(2*S1/N + cb)
                t1 = stpool.tile([128, 1], f32, tag="t1", name="t1")
                nc.vector.tensor_scalar(t1[:], S1[:], 2.0 / NEL, cb[:], OP.mult, OP.add)
                nc.vector.tensor_tensor(t1[:], t1[:], cb[:], OP.mult)
                nc.vector.tensor_scalar(st[:, 1:2], S2[:], 1.0 / NEL, None, OP.mult)
                nc.vector.tensor_tensor(st[:, 1:2], st[:, 1:2], t1[:], OP.add)
                tl["stv"] = st

            def tail_gsum(tl):
                st = tl["stv"]
                gsum = gps.tile([128, 2], f32, tag="gsum", name="gsum")
                nc.tensor.matmul(gsum[:], bones[:], st[:], start=True, stop=True)
                mgrp = stpool.tile([128, 1], f32, tag="mgrp", name="mgrp")
                nc.vector.tensor_scalar(
                    mgrp[:], gsum[:, 0:1], 1.0 / GSZ, None, OP.mult
                )
                vgrp = stpool.tile([128, 1], f32, tag="vgrp", name="vgrp")
                nc.vector.tensor_scalar(
                    vgrp[:], gsum[:, 1:2], 1.0 / GSZ, EPS, OP.mult, OP.add
                )
                msq = stpool.tile([128, 1], f32, tag="msq", name="msq")
                nc.vector.tensor_tensor(msq[:], mgrp[:], mgrp[:], OP.mult)
                nc.vector.tensor_tensor(vgrp[:], vgrp[:], msq[:], OP.subtract)
                sdev = stpool.tile([128, 1], f32, tag="sdev", name="sdev")
                nc.scalar.activation(sdev[:], vgrp[:], AF.Sqrt, bias=zeros1[:])
                inv = stpool.tile([128, 1], f32, tag="inv", name="inv")
                nc.vector.reciprocal(inv[:], sdev[:])
                Acoef = stpool.tile([128, 1], f32, tag="Ac", name="Acoef")
                nc.vector.tensor_tensor(Acoef[:], inv[:], gs[:], OP.mult)
                Bcoef = stpool.tile([128, 1], f32, tag="Bc", name="Bcoef")
                nc.vector.tensor_tensor(Bcoef[:], cb[:], mgrp[:], OP.subtract)
                nc.vector.tensor_tensor(Bcoef[:], Bcoef[:], Acoef[:], OP.mult)
                nc.vector.tensor_tensor(Bcoef[:], Bcoef[:], gbs[:], OP.add)
                tl["A"], tl["B"] = Acoef, Bcoef

            def tail_affine(tl):
                # z = A*y + B in place, whole sample (DVE 4x mode)
                y = tl["y"]
                nc.vector.tensor_scalar(
                    y[:], y[:], tl["A"][:], tl["B"][:], OP.mult, OP.add
                )

            def tail_vpool(tl):
                # vertical max over row pairs: [126,126] -> [63,126]
                y3 = tl["y"][:].rearrange("p (a b) -> p a b", b=OW)
                pv = pvpool.tile([128, PH, OW], f16, tag="pv", name="pv")
                nc.vector.tensor_tensor(
                    pv[:], y3[:, 0:OH:2, :], y3[:, 1:OH:2, :], OP.max
                )
                tl["pv"] = pv

            def tail_hpool(tl):
                # horizontal max over col pairs + clamp: [63,126] -> [63,63]
                pv = tl["pv"]
                po = popool.tile([128, PH, PW], f16, tag="po", name="po")
                nc.vector.tensor_tensor(
                    po[:], pv[:, :, 0:OW:2], pv[:, :, 1:OW:2], OP.max
                )
                nc.vector.tensor_scalar(po[:], po[:], 1.0, 0.0, OP.min, OP.max)
                tl["po"] = po

            def tail_store(tl):
                nc.sync.dma_start(
                    out_d[tl["b"]].rearrange("c h w -> c (h w)"),
                    tl["po"][:].rearrange("p a b -> p (a b)"),
                )

            # --- dead-code probes for new instruction types (bisect) ---
            PROBE = 'a'
            if PROBE:
                dprobe = cpool.tile([128, 64], f16, name="dprobe")
                nc.vector.memset(dprobe[:], 1.0)
                dacc = cpool.tile([128, 4], f32, name="dacc")
                dscr = cpool.tile([128, 64], f16, name="dscr")
                if PROBE in ('a', True):
                    nc.scalar.activation(dscr[:], dprobe[:], AF.Copy,
                                         accum_out=dacc[:, 0:1])
                if PROBE in ('b', True):
                    nc.vector.tensor_tensor_reduce(
                        dscr[:], dprobe[:], dprobe[:], 1.0, 0.0,
                        OP.mult, OP.add, dacc[:, 1:2],
                    )
                if PROBE in ('c', True):
                    nc.vector.tensor_reduce(dacc[:, 2:3], dprobe[:],
                                            mybir.AxisListType.XYZW, OP.add)
                if PROBE in ('d', True):
                    nc.vector.memset(dacc[:, 2:3], 1.0)
                    nc.vector.tensor_scalar(dacc[:, 3:4], dacc[:, 2:3],
                                            1.0 / NEL, cb[:], OP.mult, OP.add)

            pending = None
            for b in range(BPC):
                y_raw = ypool.tile([128, S], f16, tag="y", name="y_raw")
                s1cols = stpool.tile([128, NGROUPS], f32, tag="s1c", name="s1cols")
                sqcols = stpool.tile([128, NGROUPS], f32, tag="sqc", name="sqcols")
                stats = stpool.tile([128, 34, 6], f32, tag="st", name="stats")

                gi = 0  # group index within sample
                for ci, (xr0, nxr, or0, nor) in enumerate(CHUNKS):
                    # xt block0 = x rows xr0.., block1 = x rows xr0+1..;
                    # only `nor` rows each are read (kh2 taps come from xq)
                    xt = xpool.tile([128, XROWS_MAX, W], f16, tag="x", name="xt")
                    nc.sync.dma_start(
                        xt[0:64, 0:nor, :], xs[b, :, xr0 : xr0 + nor, :]
                    )
                    nc.sync.dma_start(
                        xt[64:128, 0:nor, :], xs[b, :, xr0 + 1 : xr0 + 1 + nor, :]
                    )
                    # xq: x rows (xr0+2) duplicated with a 1-column shift
                    # between partition blocks -> covers taps (kh2,kw0)+(kh2,kw1)
                    # in one 128-contraction matmul; block0 also serves (kh2,kw2)
                    xq = xqpool.tile([128, XROWS_MAX, W], f16, tag="xq", name="xq")
                    xf = xs[b].rearrange("c h w -> c (h w)")
                    off = (xr0 + 2) * W
                    nc.sync.dma_start(
                        xq[0:64, 0:nor, :].rearrange("p a b -> p (a b)"),
                        xf[:, off : off + nor * W],
                    )
                    n2 = min(nor * W, H * W - off - 1)
                    nc.sync.dma_start(
                        xq[64:128, 0:nor, :].rearrange("p a b -> p (a b)")[:, 0:n2],
                        xf[:, off + 1 : off + 1 + n2],
                    )

                    g0 = or0
                    while g0 < or0 + nor:
                        gn = min(8, or0 + nor - g0)  # 8, 6 or 4 output rows
                        hr = gn // 2  # rows per half
                        cp = cps.tile([128, 1024], f32, tag="cp", name="cp")
                        for half in range(2):
                            row0 = g0 + half * hr
                            l0 = row0 - xr0
                            outap = cp[:, half * 512 : half * 512 + hr * OW]
                            for kw in range(3):
                                nc.tensor.matmul(
                                    outap,
                                    wp[:, kw * COUT : (kw + 1) * COUT],
                                    xt[:, l0 : l0 + hr, kw : kw + OW],
                                    start=(kw == 0),
                                    stop=False,
                                )
                            nc.tensor.matmul(
                                outap,
                                w2[:],
                                xq[:, l0 : l0 + hr, 0:OW],
                                start=False,
                                stop=False,
                            )
                        # (kh2,kw2) singles for both halves, adjacent on
                        # disjoint PE row groups (0-63 / 64-127) so the
                        # 16x 32x32 sub-arrays overlap their execution.
                        # half1 reads xq block1 (data shifted +1 col) at
                        # offset 1 -> x column c+2, same tap.
                        l0a = g0 - xr0
                        l0b = g0 + hr - xr0
                        nc.tensor.matmul(
                            cp[:, 0 : hr * OW],
                            ws[0:64, :],
                            xq[0:64, l0a : l0a + hr, 2 : 2 + OW],
                            start=False,
                            stop=True,
                            skip_group_check=True,
                        )
                        nc.tensor.matmul(
                            cp[:, 512 : 512 + hr * OW],
                            ws[64:128, :],
                            xq[64:128, l0b : l0b + hr, 1 : 1 + OW],
                            start=False,
                            stop=True,
                            skip_group_check=True,
                        )
                        # evacuate both halves in one strided ACT copy;
                        # accum_out gives this group's per-channel sum(y)
                        yv = y_raw[:, g0 * OW : (g0 + gn) * OW].rearrange(
                            "p (a b) -> p a b", b=hr * OW
                        )
                        nc.scalar.activation(
                            yv,
                            cp[:].rearrange("p (a b) -> p a b", b=512)[
                                :, :, 0 : hr * OW
                            ],
                            AF.Copy,
                            accum_out=(s1cols[:, gi : gi + 1]
                                       if USE_NEW_STATS else None),
                        )
                        if USE_NEW_STATS:
                            # sum(y^2) for the group in one DVE pass
                            yseg = y_raw[:, g0 * OW : (g0 + gn) * OW]
                            nc.vector.tensor_tensor_reduce(
                                sqscr[:, 0 : gn * OW], yseg, yseg, 1.0, 0.0,
                                OP.mult, OP.add, sqcols[:, gi : gi + 1],
                            )
                        else:
                            for half in range(2):
                                r0 = (g0 + half * hr) * OW
                                nc.vector.bn_stats(
                                    stats[:, 2 * gi + half, :],
                                    y_raw[:, r0 : r0 + hr * OW],
                                )
                        gi += 1
                        g0 += gn

                    if pending is not None:
                        if ci == 0:
                            tail_stats(pending)
                        elif ci == 1:
                            tail_gsum(pending)
                        elif ci == 2:
                            tail_affine(pending)
                        elif ci == 3:
                            tail_vpool(pending)
                        elif ci == 4:
                            tail_hpool(pending)
                        elif ci == 5:
                            tail_store(pending)
                            pending = None

                pending = {"b": b, "s1c": s1cols, "sqc": sqcols, "y": y_raw,
                           "st": stats}
            tail_stats(pending)
            tail_gsum(pending)
            tail_affine(pending)
            tail_vpool(pending)
            tail_hpool(pending)
            tail_store(pending)
    nc.finalize()
    _CACHED["nc"] = nc
    return nc


def _prep_consts(conv_w, conv_b, gn_w, gn_b, scale):
    # wp[kw, ci + 64*kh, co] = conv_w[co, ci, kh, kw] for kh in {0,1}
    # w2[ci, co] = conv_w[co, ci, 2, 0]; w2[64+ci, co] = conv_w[co, ci, 2, 1]
    # ws[ci, co] = conv_w[co, ci, 2, 2]
    w = np.ascontiguousarray(conv_w.astype(np.float32))
    wp = np.empty((3, 128, COUT), np.float16)
    w2 = np.empty((128, COUT), np.float16)
    ws = np.empty((128, COUT), np.float16)
    for kw in range(3):
        wp[kw, 0:64, :] = w[:, :, 0, kw].T
        wp[kw, 64:128, :] = w[:, :, 1, kw].T
    w2[0:64, :] = w[:, :, 2, 0].T
    w2[64:128, :] = w[:, :, 2, 1].T
    ws[0:64, :] = w[:, :, 2, 2].T
    ws[64:128, :] = w[:, :, 2, 2].T
    cb = conv_b.astype(np.float32).reshape(COUT, 1)
    sc = scale.astype(np.float32).reshape(COUT)
    gs = (gn_w.astype(np.float32) * sc).reshape(COUT, 1)
    gbs = (gn_b.astype(np.float32) * sc).reshape(COUT, 1)
    bones = np.zeros((COUT, COUT), np.float32)
    for g in range(NG):
        bones[g * GSZ : (g + 1) * GSZ, g * GSZ : (g + 1) * GSZ] = 1.0
    return wp, w2, ws, cb, gs, gbs, bones


def kernel(x, conv_w, conv_b, gn_w, gn_b, scale):
    x = np.asarray(x, dtype=np.float32).astype(np.float16)
    wp, w2, ws, cb, gs, gbs, bones = _prep_consts(
        np.asarray(conv_w), np.asarray(conv_b), np.asarray(gn_w),
        np.asarray(gn_b), np.asarray(scale),
    )
    nc = _build()
    in_maps = []
    for c in range(N_CORES):
        in_maps.append({
            "xs": x[c * BPC : (c + 1) * BPC],
            "wp": wp, "w2": w2, "ws": ws,
            "cb": cb, "gs": gs, "gbs": gbs, "bones": bones,
        })
    results = _run_cached(nc, in_maps)
    out = np.concatenate([results[c]["out"] for c in range(N_CORES)], axis=0)
    return out.astype(np.float32)


def _run_cached(nc, in_maps):
    """run_bass_kernel_spmd's axon path with the jitted executable cached
    across calls (avoids re-tracing the shard_map wrapper every call)."""
    import jax
    import numpy as _np
    from jax.sharding import Mesh, PartitionSpec
    from jax.experimental.shard_map import shard_map
    from concourse import bass2jax

    if "runner" not in _CACHED:
        bass2jax.install_neuronx_cc_hook()
        partition_name = (
            nc.partition_id_tensor.name if nc.partition_id_tensor else None
        )
        in_names, out_names, out_avals, zero_outs = [], [], [], []
        for alloc in nc.m.functions[0].allocations:
            if not isinstance(alloc, mybir.MemoryLocationSet):
                continue
            name = alloc.memorylocations[0].name
            if alloc.kind == "ExternalInput":
                if name != partition_name:
                    in_names.append(name)
            elif alloc.kind == "ExternalOutput":
                shape = tuple(alloc.tensor_shape)
                dtype = mybir.dt.np(alloc.dtype)
                out_names.append(name)
                out_avals.append(jax.core.ShapedArray(shape, dtype))
                zero_outs.append(_np.zeros(shape, dtype))
        n_params = len(in_names)
        n_outs = len(out_avals)
        all_names = list(in_names) + list(out_names)
        if partition_name is not None:
            all_names.append(partition_name)
        donate = tuple(range(n_params, n_params + n_outs))

        def _body(*args):
            operands = list(args)
            if partition_name is not None:
                operands.append(bass2jax.partition_id_tensor())
            outs = bass2jax._bass_exec_p.bind(
                *operands,
                out_avals=tuple(out_avals),
                in_names=tuple(all_names),
                out_names=tuple(out_names),
                lowering_input_output_aliases=(),
                sim_require_finite=True,
                sim_require_nnan=True,
                nc=nc,
            )
            return tuple(outs)

        devices = jax.devices()[:N_CORES]
        mesh = Mesh(_np.asarray(devices), ("core",))
        in_specs = (PartitionSpec("core"),) * (n_params + n_outs)
        out_specs = (PartitionSpec("core"),) * n_outs
        sharded = jax.jit(
            shard_map(_body, mesh=mesh, in_specs=in_specs,
                      out_specs=out_specs, check_rep=False),
            donate_argnums=donate, keep_unused=True,
        )
        _CACHED["runner"] = (sharded, in_names, out_names, out_avals, zero_outs)

    sharded, in_names, out_names, out_avals, zero_outs = _CACHED["runner"]
    import numpy as _np2
    concat_in = [
        _np2.concatenate([_np2.asarray(in_maps[c][n]) for c in range(N_CORES)], axis=0)
        for n in in_names
    ]
    concat_zeros = [
        _np2.zeros((N_CORES * z.shape[0], *z.shape[1:]), z.dtype) for z in zero_outs
    ]
    out_arrs = sharded(*concat_in, *concat_zeros)
    return [
        {
            name: _np2.asarray(out_arrs[i]).reshape(N_CORES, *out_avals[i].shape)[c]
            for i, name in enumerate(out_names)
        }
        for c in range(N_CORES)
    ]


if __name__ == "__main__":
    rng = np.random.default_rng(0)
    x = rng.standard_normal((B_FULL, CIN, H, W), dtype=np.float32)
    cw = rng.standard_normal((COUT, CIN, 3, 3), dtype=np.float32)
    out = kernel(x, cw, rng.standard_normal(COUT, dtype=np.float32),
                 rng.standard_normal(COUT, dtype=np.float32),
                 rng.standard_normal(COUT, dtype=np.float32),
                 rng.standard_normal((COUT, 1, 1), dtype=np.float32))
    print(out.shape, out.dtype)


# revision 25
# speedup vs baseline: 1.1887x; 1.1887x over previous
"""Fused conv3x3 -> GroupNorm(16) -> channel scale -> maxpool2x2 -> clamp[0,1]
Trainium2 Bass kernel, data-parallel over batch on 8 NeuronCores.

Input  x [32, 64, 128, 128] f32  -> output [32, 128, 63, 63] f32.
Each core handles 4 samples.

Conv: fp16 tap-pair matmuls, 5 PE passes per 8-row output group half
(vs 9 naive):
  - xt buffer: partition ci holds x[ci, row], ci+64 holds x[ci, row+1]
    -> one [128,128] stacked weight covers taps (kh0,kw)+(kh1,kw): 3 passes
  - xq buffer: both blocks hold x[ci, row+2], block1 shifted 1 column
    (loaded as flat row-major slices at +0/+1 element offsets, so both are
    single contiguous DMAs) -> taps (kh2,0)+(kh2,1) in 1 pass; (kh2,2)
    reads xq block0 at column offset 2 as a 64-contraction pass.
The two 64-contraction passes land on disjoint PE row groups so their
execution overlaps; PE busy ~147us/core is within ~15% of the MAC roofline.

Tail strategy (keeps the PE fed; DVE/ACT stay well under the PE's
~37us/sample):
  - GroupNorm stats as raw sums, no bn_stats: the ACT PSUM-evacuation Copy
    produces per-group sum(y) via accum_out; one DVE scalar_tensor_tensor
    per group (out=(y*1)*y -> scratch, accum_out=sum) produces sum(y^2) in
    a single 2x-mode pass.  (tensor_tensor_reduce compiles but dies at
    runtime on this toolchain; scalar_tensor_tensor's accum works.)
    Conv bias is folded analytically into the final per-channel affine.
  - the 8-channel group reduction is a tiny block-diagonal-ones matmul kept
    in fp16 (fp32r matmuls flush the PE pipeline on mode switch).
  - affine BEFORE pooling, as a single in-place DVE tensor_scalar over the
    whole sample -- contiguous fp16 SBUF operands hit the DVE 4x perf mode.
  - single max-pool branch, vertical pairs first (contiguous innermost ->
    DVE 2x mode), then horizontal pairs (strided, 1x), then fused clamp.
  - per-sample tails (coeff chain / affine / pools / store) are emitted
    interleaved with the next sample's conv chunks so no queue head-blocks.
  - output stores dispatch from the idle GpSimd SWDGE: on either hardware
    DGE queue (Sync carries x-loads, ACT carries PSUM evacs) the store's
    wait on the DVE clamp would head-block work the PE depends on.
fp16 output upcast to f32 on host.
"""

import numpy as np

import concourse.bacc as bacc
import concourse.mybir as mybir
import concourse.tile as tile
from concourse.bass_utils import run_bass_kernel_spmd

N_CORES = 8
B_FULL, CIN, H, W = 32, 64, 128, 128
COUT = 128
BPC = B_FULL // N_CORES  # samples per core
OH = OW = 126
PH = PW = 63
NG = 16  # groups
GSZ = COUT // NG  # 8 channels per group
EPS = 1e-5
S = OH * OW  # spatial size per sample
NEL = float(S)  # elements per channel for stats

# (x_row0, n_xrows, out_row0, n_out_rows)
CHUNKS = [(0, 10, 0, 8), (8, 10, 8, 8), (16, 10, 16, 8), (24, 14, 24, 12),
          (36, 18, 36, 16), (52, 24, 52, 22), (74, 24, 74, 22), (96, 24, 96, 22),
          (118, 10, 118, 8)]
XROWS_MAX = 24
NGROUPS = 17  # total 8-or-smaller row groups per sample

_CACHED = {}


def _build():
    if "nc" in _CACHED:
        return _CACHED["nc"]
    f32 = mybir.dt.float32
    f16 = mybir.dt.float16
    AF = mybir.ActivationFunctionType
    OP = mybir.AluOpType

    nc = bacc.Bacc("TRN2", target_bir_lowering=False, debug=False)
    xs = nc.dram_tensor("xs", [BPC, CIN, H, W], f16, kind="ExternalInput").ap()
    wp_d = nc.dram_tensor("wp", [128, 3 * COUT], f16, kind="ExternalInput").ap()
    w2_d = nc.dram_tensor("w2", [128, COUT], f16, kind="ExternalInput").ap()
    ws_d = nc.dram_tensor("ws", [128, COUT], f16, kind="ExternalInput").ap()
    cb_d = nc.dram_tensor("cb", [COUT, 1], f32, kind="ExternalInput").ap()
    gs_d = nc.dram_tensor("gs", [COUT, 1], f32, kind="ExternalInput").ap()
    gbs_d = nc.dram_tensor("gbs", [COUT, 1], f32, kind="ExternalInput").ap()
    bones_d = nc.dram_tensor("bones", [COUT, COUT], f16, kind="ExternalInput").ap()
    out_d = nc.dram_tensor("out", [BPC, COUT, PH, PW], f16, kind="ExternalOutput").ap()

    with tile.TileContext(nc) as tc:
        with (
            tc.tile_pool(name="consts", bufs=1) as cpool,
            tc.tile_pool(name="xpool", bufs=3) as xpool,
            tc.tile_pool(name="xqpool", bufs=3) as xqpool,
            tc.tile_pool(name="ypool", bufs=2) as ypool,
            tc.tile_pool(name="sqpool", bufs=1) as sqpool,
            tc.tile_pool(name="stpool", bufs=2) as stpool,
            tc.tile_pool(name="pvpool", bufs=2) as pvpool,
            tc.tile_pool(name="popool", bufs=2) as popool,
            tc.tile_pool(name="cps", bufs=3, space="PSUM") as cps,
            tc.tile_pool(name="gps", bufs=1, space="PSUM") as gps,
        ):
            wp = cpool.tile([128, 3 * COUT], f16, name="wp_t")
            w2 = cpool.tile([128, COUT], f16, name="w2_t")
            ws = cpool.tile([128, COUT], f16, name="ws_t")
            cb = cpool.tile([COUT, 1], f32, name="cb_t")
            gs = cpool.tile([COUT, 1], f32, name="gs_t")
            gbs = cpool.tile([COUT, 1], f32, name="gbs_t")
            bones = cpool.tile([COUT, COUT], f16, name="bones_t")
            zeros1 = cpool.tile([COUT, 1], f32, name="zeros1")
            nc.vector.memset(zeros1[:], 0.0)
            # scratch for the y^2 main output (discarded; only accum is used)
            sqscr = sqpool.tile([128, 8 * OW], f16, name="sqscr")

            def load_chunk(b, xr0, nor):
                xt = xpool.tile([128, XROWS_MAX, W], f16, tag="x", name="xt")
                nc.sync.dma_start(
                    xt[0:64, 0:nor, :], xs[b, :, xr0 : xr0 + nor, :]
                )
                nc.sync.dma_start(
                    xt[64:128, 0:nor, :], xs[b, :, xr0 + 1 : xr0 + 1 + nor, :]
                )
                xq = xqpool.tile([128, XROWS_MAX, W], f16, tag="xq", name="xq")
                xf = xs[b].rearrange("c h w -> c (h w)")
                off = (xr0 + 2) * W
                nc.sync.dma_start(
                    xq[0:64, 0:nor, :].rearrange("p a b -> p (a b)"),
                    xf[:, off : off + nor * W],
                )
                n2 = min(nor * W, H * W - off - 1)
                nc.sync.dma_start(
                    xq[64:128, 0:nor, :].rearrange("p a b -> p (a b)")[:, 0:n2],
                    xf[:, off + 1 : off + 1 + n2],
                )
                return xt, xq

            # first matmul needs wp + chunk-0 x: issue those DMAs first, the
            # remaining consts (not needed until later matmuls / tails) after
            nc.sync.dma_start(wp[:], wp_d[:])
            prefetch = load_chunk(0, CHUNKS[0][0], CHUNKS[0][3])
            nc.sync.dma_start(w2[:], w2_d[:])
            nc.sync.dma_start(ws[:], ws_d[:])
            nc.sync.dma_start(cb[:], cb_d[:])
            nc.sync.dma_start(gs[:], gs_d[:])
            nc.sync.dma_start(gbs[:], gbs_d[:])
            nc.sync.dma_start(bones[:], bones_d[:])

            def tail_stats(tl):
                # full-sample st0 = E[y+cb], st1 = E[(y+cb)^2] from raw sums,
                # cast to fp16 for the group-reduction matmul
                S1 = stpool.tile([128, 1], f32, tag="S1", name="S1")
                nc.vector.tensor_reduce(S1[:], tl["s1c"][:],
                                        mybir.AxisListType.XYZW, OP.add)
                S2 = stpool.tile([128, 1], f32, tag="S2", name="S2")
                nc.vector.tensor_reduce(S2[:], tl["sqc"][:],
                                        mybir.AxisListType.XYZW, OP.add)
                st = stpool.tile([128, 2], f32, tag="sts", name="st")
                # st0 = S1/N + cb ; st1 = S2/N + cb*(2*S1/N + cb)
                nc.vector.tensor_scalar(st[:, 0:1], S1[:], 1.0 / NEL, cb[:],
                                        OP.mult, OP.add)
                t1 = stpool.tile([128, 1], f32, tag="t1", name="t1")
                nc.vector.tensor_scalar(t1[:], S1[:], 2.0 / NEL, cb[:],
                                        OP.mult, OP.add)
                nc.vector.tensor_tensor(t1[:], t1[:], cb[:], OP.mult)
                nc.vector.scalar_tensor_tensor(st[:, 1:2], S2[:], 1.0 / NEL,
                                               t1[:], OP.mult, OP.add)
                stf = stpool.tile([128, 2], f16, tag="stf", name="stf")
                nc.vector.tensor_scalar(stf[:], st[:], 1.0, None, OP.mult)
                tl["stv"] = stf

            def tail_gsum(tl):
                stf = tl["stv"]
                gsum = gps.tile([128, 2], f32, tag="gsum", name="gsum")
                nc.tensor.matmul(gsum[:], bones[:], stf[:], start=True, stop=True)
                mgrp = stpool.tile([128, 1], f32, tag="mgrp", name="mgrp")
                nc.vector.tensor_scalar(
                    mgrp[:], gsum[:, 0:1], 1.0 / GSZ, None, OP.mult
                )
                vgrp = stpool.tile([128, 1], f32, tag="vgrp", name="vgrp")
                nc.vector.tensor_scalar(
                    vgrp[:], gsum[:, 1:2], 1.0 / GSZ, EPS, OP.mult, OP.add
                )
                msq = stpool.tile([128, 1], f32, tag="msq", name="msq")
                nc.vector.tensor_tensor(msq[:], mgrp[:], mgrp[:], OP.mult)
                nc.vector.tensor_tensor(vgrp[:], vgrp[:], msq[:], OP.subtract)
                sdev = stpool.tile([128, 1], f32, tag="sdev", name="sdev")
                nc.scalar.activation(sdev[:], vgrp[:], AF.Sqrt, bias=zeros1[:])
                inv = stpool.tile([128, 1], f32, tag="inv", name="inv")
                nc.vector.reciprocal(inv[:], sdev[:])
                Acoef = stpool.tile([128, 1], f32, tag="Ac", name="Acoef")
                nc.vector.tensor_tensor(Acoef[:], inv[:], gs[:], OP.mult)
                # B = (cb - mgrp)*A + gbs
                Bcoef = stpool.tile([128, 1], f32, tag="Bc", name="Bcoef")
                nc.vector.scalar_tensor_tensor(Bcoef[:], cb[:], mgrp[:],
                                               Acoef[:], OP.subtract, OP.mult)
                nc.vector.tensor_tensor(Bcoef[:], Bcoef[:], gbs[:], OP.add)
                tl["A"], tl["B"] = Acoef, Bcoef

            def tail_affine(tl):
                # z = A*y + B in place, whole sample (DVE 4x mode)
                y = tl["y"]
                nc.vector.tensor_scalar(
                    y[:], y[:], tl["A"][:], tl["B"][:], OP.mult, OP.add
                )

            def tail_vpool(tl):
                # vertical max over row pairs: [126,126] -> [63,126]
                y3 = tl["y"][:].rearrange("p (a b) -> p a b", b=OW)
                pv = pvpool.tile([128, PH, OW], f16, tag="pv", name="pv")
                nc.vector.tensor_tensor(
                    pv[:], y3[:, 0:OH:2, :], y3[:, 1:OH:2, :], OP.max
                )
                tl["pv"] = pv

            def tail_hpool(tl):
                # horizontal max over col pairs + clamp: [63,126] -> [63,63]
                pv = tl["pv"]
                po = popool.tile([128, PH, PW], f16, tag="po", name="po")
                nc.vector.tensor_tensor(
                    po[:], pv[:, :, 0:OW:2], pv[:, :, 1:OW:2], OP.max
                )
                nc.vector.tensor_scalar(po[:], po[:], 1.0, 0.0, OP.min, OP.max)
                tl["po"] = po

            def tail_store(tl):
                nc.gpsimd.dma_start(
                    out_d[tl["b"]].rearrange("c h w -> c (h w)"),
                    tl["po"][:].rearrange("p a b -> p (a b)"),
                )

            pending = None
            for b in range(BPC):
                y_raw = ypool.tile([128, S], f16, tag="y", name="y_raw")
                s1cols = stpool.tile([128, NGROUPS], f32, tag="s1c", name="s1cols")
                sqcols = stpool.tile([128, NGROUPS], f32, tag="sqc", name="sqcols")

                gi = 0  # group index within sample
                for ci, (xr0, nxr, or0, nor) in enumerate(CHUNKS):
                    # xt block0 = x rows xr0.., block1 = x rows xr0+1..;
                    # only `nor` rows each are read (kh2 taps come from xq).
                    # xq holds x rows (xr0+2) duplicated with a 1-column shift
                    # between partition blocks -> covers taps (kh2,kw0)+(kh2,kw1)
                    # in one 128-contraction matmul; block0 also serves (kh2,kw2)
                    if b == 0 and ci == 0:
                        xt, xq = prefetch
                    else:
                        xt, xq = load_chunk(b, xr0, nor)

                    g0 = or0
                    while g0 < or0 + nor:
                        gn = min(8, or0 + nor - g0)  # 8, 6 or 4 output rows
                        hr = gn // 2  # rows per half
                        cp = cps.tile([128, 1024], f32, tag="cp", name="cp")
                        for half in range(2):
                            row0 = g0 + half * hr
                            l0 = row0 - xr0
                            outap = cp[:, half * 512 : half * 512 + hr * OW]
                            for kw in range(3):
                                nc.tensor.matmul(
                                    outap,
                                    wp[:, kw * COUT : (kw + 1) * COUT],
                                    xt[:, l0 : l0 + hr, kw : kw + OW],
                                    start=(kw == 0),
                                    stop=False,
                                )
                            nc.tensor.matmul(
                                outap,
                                w2[:],
                                xq[:, l0 : l0 + hr, 0:OW],
                                start=False,
                                stop=False,
                            )
                        # (kh2,kw2) singles for both halves, adjacent on
                        # disjoint PE row groups (0-63 / 64-127) so the
                        # 16x 32x32 sub-arrays overlap their execution.
                        # half1 reads xq block1 (data shifted +1 col) at
                        # offset 1 -> x column c+2, same tap.
                        l0a = g0 - xr0
                        l0b = g0 + hr - xr0
                        nc.tensor.matmul(
                            cp[:, 0 : hr * OW],
                            ws[0:64, :],
                            xq[0:64, l0a : l0a + hr, 2 : 2 + OW],
                            start=False,
                            stop=True,
                            skip_group_check=True,
                        )
                        nc.tensor.matmul(
                            cp[:, 512 : 512 + hr * OW],
                            ws[64:128, :],
                            xq[64:128, l0b : l0b + hr, 1 : 1 + OW],
                            start=False,
                            stop=True,
                            skip_group_check=True,
                        )
                        # evacuate both halves in one strided ACT copy;
                        # accum_out gives this group's per-channel sum(y)
                        yv = y_raw[:, g0 * OW : (g0 + gn) * OW].rearrange(
                            "p (a b) -> p a b", b=hr * OW
                        )
                        nc.scalar.activation(
                            yv,
                            cp[:].rearrange("p (a b) -> p a b", b=512)[
                                :, :, 0 : hr * OW
                            ],
                            AF.Copy,
                            accum_out=s1cols[:, gi : gi + 1],
                        )
                        # sum(y^2) in one DVE pass: out=(y*1)*y -> scratch,
                        # accum_out does the add-reduce
                        yseg = y_raw[:, g0 * OW : (g0 + gn) * OW]
                        nc.vector.scalar_tensor_tensor(
                            sqscr[:, 0 : gn * OW], yseg, 1.0, yseg,
                            OP.mult, OP.mult,
                            accum_out=sqcols[:, gi : gi + 1],
                        )
                        gi += 1
                        g0 += gn

                    if pending is not None:
                        if ci == 0:
                            tail_stats(pending)
                        elif ci == 1:
                            tail_gsum(pending)
                        elif ci == 2:
                            tail_affine(pending)
                        elif ci == 3:
                            tail_vpool(pending)
                        elif ci == 4:
                            tail_hpool(pending)
                        elif ci == 6:
                            tail_store(pending)
                            pending = None

                pending = {"b": b, "s1c": s1cols, "sqc": sqcols, "y": y_raw}
            tail_stats(pending)
            tail_gsum(pending)
            tail_affine(pending)
            tail_vpool(pending)
            tail_hpool(pending)
            tail_store(pending)
    nc.finalize()
    _CACHED["nc"] = nc
    return nc


def _prep_consts(conv_w, conv_b, gn_w, gn_b, scale):
    # wp[ci + 64*kh, kw*COUT + co] = conv_w[co, ci, kh, kw] for kh in {0,1}
    # w2[ci, co] = conv_w[co, ci, 2, 0]; w2[64+ci, co] = conv_w[co, ci, 2, 1]
    # ws[ci, co] = conv_w[co, ci, 2, 2]
    w = np.ascontiguousarray(conv_w.astype(np.float32))
    wp = np.empty((128, 3 * COUT), np.float16)
    w2 = np.empty((128, COUT), np.float16)
    ws = np.empty((128, COUT), np.float16)
    for kw in range(3):
        wp[0:64, kw * COUT : (kw + 1) * COUT] = w[:, :, 0, kw].T
        wp[64:128, kw * COUT : (kw + 1) * COUT] = w[:, :, 1, kw].T
    w2[0:64, :] = w[:, :, 2, 0].T
    w2[64:128, :] = w[:, :, 2, 1].T
    ws[0:64, :] = w[:, :, 2, 2].T
    ws[64:128, :] = w[:, :, 2, 2].T
    cb = conv_b.astype(np.float32).reshape(COUT, 1)
    sc = scale.astype(np.float32).reshape(COUT)
    gs = (gn_w.astype(np.float32) * sc).reshape(COUT, 1)
    gbs = (gn_b.astype(np.float32) * sc).reshape(COUT, 1)
    bones = np.zeros((COUT, COUT), np.float16)
    for g in range(NG):
        bones[g * GSZ : (g + 1) * GSZ, g * GSZ : (g + 1) * GSZ] = 1.0
    return wp, w2, ws, cb, gs, gbs, bones


def kernel(x, conv_w, conv_b, gn_w, gn_b, scale):
    x = np.asarray(x, dtype=np.float32).astype(np.float16)
    wp, w2, ws, cb, gs, gbs, bones = _prep_consts(
        np.asarray(conv_w), np.asarray(conv_b), np.asarray(gn_w),
        np.asarray(gn_b), np.asarray(scale),
    )
    nc = _build()
    in_maps = []
    for c in range(N_CORES):
        in_maps.append({
            "xs": x[c * BPC : (c + 1) * BPC],
            "wp": wp, "w2": w2, "ws": ws,
            "cb": cb, "gs": gs, "gbs": gbs, "bones": bones,
        })
    results = _run_cached(nc, in_maps)
    out = np.concatenate([results[c]["out"] for c in range(N_CORES)], axis=0)
    return out.astype(np.float32)


def _run_cached(nc, in_maps):
    """run_bass_kernel_spmd's axon path with the jitted executable cached
    across calls (avoids re-tracing the shard_map wrapper every call)."""
    import jax
    import numpy as _np
    from jax.sharding import Mesh, PartitionSpec
    from jax.experimental.shard_map import shard_map
    from concourse import bass2jax

    if "runner" not in _CACHED:
        bass2jax.install_neuronx_cc_hook()
        partition_name = (
            nc.partition_id_tensor.name if nc.partition_id_tensor else None
        )
        in_names, out_names, out_avals, zero_outs = [], [], [], []
        for alloc in nc.m.functions[0].allocations:
            if not isinstance(alloc, mybir.MemoryLocationSet):
                continue
            name = alloc.memorylocations[0].name
            if alloc.kind == "ExternalInput":
                if name != partition_name:
                    in_names.append(name)
            elif alloc.kind == "ExternalOutput":
                shape = tuple(alloc.tensor_shape)
                dtype = mybir.dt.np(alloc.dtype)
                out_names.append(name)
                out_avals.append(jax.core.ShapedArray(shape, dtype))
                zero_outs.append(_np.zeros(shape, dtype))
        n_params = len(in_names)
        n_outs = len(out_avals)
        all_names = list(in_names) + list(out_names)
        if partition_name is not None:
            all_names.append(partition_name)
        donate = tuple(range(n_params, n_params + n_outs))

        def _body(*args):
            operands = list(args)
            if partition_name is not None:
                operands.append(bass2jax.partition_id_tensor())
            outs = bass2jax._bass_exec_p.bind(
                *operands,
                out_avals=tuple(out_avals),
                in_names=tuple(all_names),
                out_names=tuple(out_names),
                lowering_input_output_aliases=(),
                sim_require_finite=True,
                sim_require_nnan=True,
                nc=nc,
            )
            return tuple(outs)

        devices = jax.devices()[:N_CORES]
        mesh = Mesh(_np.asarray(devices), ("core",))
        in_specs = (PartitionSpec("core"),) * (n_params + n_outs)
        out_specs = (PartitionSpec("core"),) * n_outs
        sharded = jax.jit(
            shard_map(_body, mesh=mesh, in_specs=in_specs,
                      out_specs=out_specs, check_rep=False),
            donate_argnums=donate, keep_unused=True,
        )
        _CACHED["runner"] = (sharded, in_names, out_names, out_avals, zero_outs)

    sharded, in_names, out_names, out_avals, zero_outs = _CACHED["runner"]
    import numpy as _np2
    concat_in = [
        _np2.concatenate([_np2.asarray(in_maps[c][n]) for c in range(N_CORES)], axis=0)
        for n in in_names
    ]
    concat_zeros = [
        _np2.zeros((N_CORES * z.shape[0], *z.shape[1:]), z.dtype) for z in zero_outs
    ]
    out_arrs = sharded(*concat_in, *concat_zeros)
    return [
        {
            name: _np2.asarray(out_arrs[i]).reshape(N_CORES, *out_avals[i].shape)[c]
            for i, name in enumerate(out_names)
        }
        for c in range(N_CORES)
    ]


if __name__ == "__main__":
    rng = np.random.default_rng(0)
    x = rng.standard_normal((B_FULL, CIN, H, W), dtype=np.float32)
    cw = rng.standard_normal((COUT, CIN, 3, 3), dtype=np.float32)
    out = kernel(x, cw, rng.standard_normal(COUT, dtype=np.float32),
                 rng.standard_normal(COUT, dtype=np.float32),
                 rng.standard_normal(COUT, dtype=np.float32),
                 rng.standard_normal((COUT, 1, 1), dtype=np.float32))
    print(out.shape, out.dtype)
